# revision 24
# baseline (speedup 1.0000x reference)
"""Causal multi-head attention with relative position bias on 8 Trainium2
NeuronCores.

Problem (full shapes): x[2,2048,1024], rel_bias[16,2048,2048],
w_qkv[1024,3072], b_qkv[3072], w_out[1024,1024], b_out[1024].

Sharding: core = (batch, head-group): 2 batches x 4 head-groups of 4 heads.
Each core computes q/k/v projections for its 4 heads, causal attention with
rel-bias, and a partial output projection through its heads' rows of w_out.
Host sums the 4 partial outputs per batch (the tensor-parallel reduce) and
adds b_out.

Device kernel design notes:
- Scores are computed TRANSPOSED (scoresT[kj,qi] = k.q) so no on-chip
  transposes are needed anywhere: softmax reduction over keys becomes a
  matmul contraction, handled by appending a ones-column to V; the PV matmul
  directly produces the transposed attention output that the out-projection
  needs as its stationary operand.
- exp(score + bias) = exp(score) * exp(bias): host precomputes exp(rel_biasT)
  in bf16 with the causal mask baked in as exact zeros. ACT does a pure exp
  straight from PSUM; DVE multiplies two bf16 SBUF operands at 2x rate.
- Queries are processed in 512-wide windows (4 per core); each window's
  score/exp/mul/PV tiles are trimmed at the causal diagonal.
- Both heads of a pair share one [128,2,512] score PSUM tile so exp and the
  erb multiply are ONE instruction per (window, key-block) - halving the
  per-instruction overhead on ACT and DVE.
- The PE instruction stream is statically interleaved: qk/v projection units
  and out-projection units are emitted as fillers inside the attention
  windows so the tensor engine never idles (keeps the HAM clock gate at
  2.4 GHz) while ACT/DVE drain the softmax work.
- Normalization is deferred and batched: unnormalized PV output is staged to
  SBUF (f32), denominator rows are DMA-gathered into a [4,512] tile, ONE
  reciprocal_approx_fast computes 1/den for the whole window round, a pair of
  K=1 matmuls broadcasts it across partitions, and DVE applies it while the
  out-projection of the previous round runs on the PE.
"""

import math
import sys
import types
from collections import deque
from contextlib import ExitStack

import ml_dtypes
import numpy as np

B, S, D = 2, 2048, 1024
NH, HD = 16, 64
NCORES = 8
HPC = 4  # heads per core (2 pairs)

_BF16 = ml_dtypes.bfloat16

KC = D // 128   # 8 contraction chunks for the projections
NW = S // 512   # 4 query windows
NSC = S // 128  # 16 s-chunks


def _install_ntff_hook():
    """concourse.bass_utils imports antenv.axon_hooks for NTFF tracing under
    axon; this container's antenv lacks that module. Provide it, backed by
    the ctypes hook from trn_agent_boot (if present)."""
    if "antenv.axon_hooks" in sys.modules:
        return
    try:
        import antenv
    except ImportError:
        return
    mod = types.ModuleType("antenv.axon_hooks")
    mod._hook = None
    mod.set_axon_ntff_profile_hook = lambda h: setattr(mod, "_hook", h)
    mod.get_axon_ntff_profile_hook = lambda: mod._hook
    sys.modules["antenv.axon_hooks"] = mod
    antenv.axon_hooks = mod
    try:
        from trn_agent_boot.trn_boot import _ntff_profile_via_ctypes

        h = _ntff_profile_via_ctypes("/opt/axon/libaxon_pjrt.so")
        if h is not None:
            mod._hook = h
    except Exception:
        pass


_LDW_OPT_INSTALLED = False


def _enable_ldw_opt():
    """walrus ships with --enable-ldw-opt=false; flip it for this process
    (dedupes/hoists LDWEIGHTS). Gated by KERNEL_LDW_OPT=1."""
    global _LDW_OPT_INSTALLED
    if _LDW_OPT_INSTALLED:
        return
    _LDW_OPT_INSTALLED = True
    import os
    if os.environ.get("KERNEL_LDW_OPT", "0") != "1":
        return
    import concourse.bass_utils as bu
    orig = bu.run_command

    def patched(argv, **kwargs):
        argv = ["--enable-ldw-opt=true" if a == "--enable-ldw-opt=false" else a
                for a in argv]
        return orig(argv, **kwargs)

    bu.run_command = patched


def _build_program(has_bqk: bool, has_bv: bool):
    import concourse.tile as tile
    from concourse import bacc, mybir

    bf = mybir.dt.bfloat16
    f32 = mybir.dt.float32
    EXP = mybir.ActivationFunctionType.Exp

    nc = bacc.Bacc("TRN2", target_bir_lowering=False, debug=False,
                   num_devices=NCORES)

    d = types.SimpleNamespace()
    d.xT = nc.dram_tensor("xT", [D, S], bf, kind="ExternalInput").ap()
    d.wqk = nc.dram_tensor("wqk", [D, 512], bf, kind="ExternalInput").ap()
    d.wv = nc.dram_tensor("wv", [D, 260], bf, kind="ExternalInput").ap()
    d.bqk = nc.dram_tensor("bqk", [4, 128], bf, kind="ExternalInput").ap()
    d.bv = nc.dram_tensor("bv", [1, 260], bf, kind="ExternalInput").ap()
    d.erb = nc.dram_tensor("erb", [HPC, S, S], bf, kind="ExternalInput").ap()
    d.wo = nc.dram_tensor("wo", [2, 128, D], bf, kind="ExternalInput").ap()
    d.out = nc.dram_tensor("out", [S, D], bf, kind="ExternalOutput").ap()

    # erb source view: p(row within key block), c(key block), h(local head), q
    erb_v = d.erb.rearrange("h (c p) q -> p c h q", p=128)

    st = types.SimpleNamespace()

    with tile.TileContext(nc) as tc, ExitStack() as ctx:
        # ---------------- SBUF pools ----------------
        xt_pool = ctx.enter_context(tc.tile_pool(name="xt", bufs=1))
        wqk_pool = ctx.enter_context(tc.tile_pool(name="wqk", bufs=1))
        wv_pool = ctx.enter_context(tc.tile_pool(name="wv", bufs=1))
        wo_pool = ctx.enter_context(tc.tile_pool(name="wo", bufs=1))
        const_pool = ctx.enter_context(tc.tile_pool(name="consts", bufs=1))
        qkT_pool = ctx.enter_context(tc.tile_pool(name="qkT", bufs=4))
        v_pool = ctx.enter_context(tc.tile_pool(name="vsb", bufs=NSC))
        attnT_pool = ctx.enter_context(tc.tile_pool(name="attnT", bufs=2))
        erb_pool = ctx.enter_context(tc.tile_pool(name="erb", bufs=13))
        esc_pool = ctx.enter_context(tc.tile_pool(name="esc", bufs=4))
        pr_pool = ctx.enter_context(tc.tile_pool(name="prob", bufs=5))
        pvu_pool = ctx.enter_context(tc.tile_pool(name="pvu", bufs=6))
        den_pool = ctx.enter_context(tc.tile_pool(name="den", bufs=2))
        osb_pool = ctx.enter_context(tc.tile_pool(name="osb", bufs=4))
        # ---------------- PSUM pools ----------------
        sc_ps = ctx.enter_context(tc.tile_pool(name="sc_ps", bufs=2, space="PSUM"))
        pv_ps = ctx.enter_context(tc.tile_pool(name="pv_ps", bufs=2, space="PSUM"))
        aux_ps = ctx.enter_context(tc.tile_pool(name="aux_ps", bufs=2, space="PSUM"))

        # ---------------- constants ----------------
        ones_row = const_pool.tile([1, 512], bf)
        nc.gpsimd.memset(ones_row[:], 1.0)
        ones64 = const_pool.tile([1, 64], bf)
        nc.gpsimd.memset(ones64[:], 1.0)

        # ---------------- input DMAs (consolidated, critical-path first) ----
        xT_v = d.xT.rearrange("(k p) q -> p k q", p=128)
        wqk_v = d.wqk.rearrange("(k p) c -> p k c", p=128)
        wv_v = d.wv.rearrange("(k p) c -> p k c", p=128)
        wo_v = d.wo.rearrange("w p d -> p w d")

        wqk_big = wqk_pool.tile([128, KC, 512], bf)
        nc.sync.dma_start(wqk_big[:, :, 0:256], wqk_v[:, :, 0:256])
        st.wqk_t = [wqk_big[:, k, :] for k in range(KC)]
        xt_big = xt_pool.tile([128, KC, S], bf)
        nc.sync.dma_start(xt_big[:, 0:4, 0:512], xT_v[:, 0:4, 0:512])
        nc.sync.dma_start(xt_big[:, 4:KC, 0:512], xT_v[:, 4:KC, 0:512])
        nc.sync.dma_start(wqk_big[:, :, 256:512], wqk_v[:, :, 256:512])
        st.xt_t = [xt_big[:, k, :] for k in range(KC)]
        wv_big = wv_pool.tile([128, KC, 260], bf)
        nc.sync.dma_start(wv_big[:], wv_v[:])
        st.wv_t = [wv_big[:, k, :] for k in range(KC)]

        def mk_xt_dma(s4):
            def emit():
                nc.sync.dma_start(
                    xt_big[:, :, s4 * 512:(s4 + 1) * 512],
                    xT_v[:, :, s4 * 512:(s4 + 1) * 512])
            return emit

        wo_big = wo_pool.tile([128, 2, D], bf)
        st.wo_t = [wo_big[:, p, :] for p in range(2)]

        def mk_wo_dma():
            def emit():
                nc.sync.dma_start(wo_big[:], wo_v[:])
            return emit

        if has_bqk:
            st.bqk_sb = []
            for m in range(4):
                t = const_pool.tile([1, 128], bf, name=f"bqk{m}", tag=f"bqk{m}")
                nc.sync.dma_start(t[:], d.bqk[m:m + 1, :])
                st.bqk_sb.append(t)
        if has_bv:
            st.bv_sb = const_pool.tile([1, 260], bf)
            nc.sync.dma_start(st.bv_sb[:], d.bv[:])

        st.qkT_t = [qkT_pool.tile([128, S], bf, name=f"qkT{m}", tag="qkT")
                    for m in range(4)]
        st.v_t = [v_pool.tile([128, 260], bf, name=f"v{si}", tag="vsb")
                  for si in range(NSC)]
        st.attnT_t = [attnT_pool.tile([128, S], bf, name=f"attnT{p}", tag="attnT")
                      for p in range(2)]

        # ---------------- erb chunk prefetch machinery ----------------
        # chunk list in consumption order; each covers 2 key-blocks x 2 heads
        erb_tiles = {}

        def erb_chunk_key(p, w, c):
            return (p, w, c)

        erb_sched = []  # (key, emit_fn)

        def mk_erb_emit(p, w, c):
            def emit():
                last = (c == 2 * w + 1)
                qoff = 256 if last else 0
                wd = 512 - qoff
                t = erb_pool.tile([128, 2, 2, 512], bf, name="erbt", tag="erbt")
                for h in range(2):
                    nc.sync.dma_start(
                        t[:, :, h, 0:wd],
                        erb_v[:, 2 * c:2 * c + 2, 2 * p + h,
                              512 * w + qoff:512 * (w + 1)])
                erb_tiles[(p, w, c)] = (t, qoff)
            return emit

        for w in range(NW):
            for p in range(2):
                for c in range(2 * w + 2):
                    erb_sched.append(((p, w, c), mk_erb_emit(p, w, c)))
        st.erb_emitted = 0

        ERB_LEAD = 10

        def pump_erb(consumed):
            # keep ERB_LEAD chunks issued ahead of the consumption cursor
            while (st.erb_emitted < len(erb_sched)
                   and st.erb_emitted < consumed + ERB_LEAD):
                erb_sched[st.erb_emitted][1]()
                st.erb_emitted += 1

        # ---------------- filler units ----------------
        def mk_qk_unit(m, s4):
            def emit():
                ps = aux_ps.tile([128, 512], f32, name="qkps", tag="aux")
                for k in range(KC):
                    nc.tensor.matmul(
                        ps[:],
                        st.wqk_t[k][:, m * 128:(m + 1) * 128],
                        st.xt_t[k][:, s4 * 512:(s4 + 1) * 512],
                        start=(k == 0),
                        stop=(k == KC - 1 and not has_bqk),
                    )
                if has_bqk:
                    nc.tensor.matmul(
                        ps[:], st.bqk_sb[m][:], ones_row[:, :],
                        start=False, stop=True,
                    )
                nc.vector.tensor_copy(
                    st.qkT_t[m][:, s4 * 512:(s4 + 1) * 512], ps[:])
            return emit

        def mk_v_unit(si):
            def emit():
                ps = aux_ps.tile([128, 512], f32, name="vps", tag="aux")
                for k in range(KC):
                    nc.tensor.matmul(
                        ps[:, 0:260],
                        st.xt_t[k][:, si * 128:(si + 1) * 128],
                        st.wv_t[k][:],
                        start=(k == 0),
                        stop=(k == KC - 1 and not has_bv),
                    )
                if has_bv:
                    nc.tensor.matmul(
                        ps[:, 0:260], ones_row[0:1, 0:128], st.bv_sb[:],
                        start=False, stop=True,
                    )
                nc.vector.tensor_copy(st.v_t[si][:], ps[:, 0:260])
                for h in range(HPC):
                    nc.gpsimd.memset(
                        st.v_t[si][:, 65 * h + 64:65 * h + 65], 1.0)
            return emit

        def mk_out_unit(si, e2, copy_eng="act"):
            def emit():
                ps = aux_ps.tile([128, 512], f32, name="ops", tag="aux")
                for p in range(2):
                    nc.tensor.matmul(
                        ps[:],
                        st.attnT_t[p][:, si * 128:(si + 1) * 128],
                        st.wo_t[p][:, e2 * 512:(e2 + 1) * 512],
                        start=(p == 0), stop=(p == 1),
                    )
                osb = osb_pool.tile([128, 512], bf, name="osb", tag="osb")
                if copy_eng == "act":
                    nc.scalar.copy(osb[:], ps[:])
                else:
                    nc.vector.tensor_copy(osb[:], ps[:])
                nc.sync.dma_start(
                    d.out[si * 128:(si + 1) * 128, e2 * 512:(e2 + 1) * 512],
                    osb[:])
            return emit

        # ---------------- attention window ----------------
        def emit_window(p, w, fillers, hooks=None):
            """One (pair, 512-query-window): scoresT -> exp -> *erb -> PV.
            `fillers`: list of closures to spread across the kj loop.
            `hooks`: dict kj -> closure, run after that kj's filler drain."""
            qT = st.qkT_t[2 * p]
            kT = st.qkT_t[2 * p + 1]
            nkj = 4 * w + 4
            q0 = 512 * w
            pv = [pv_ps.tile([65, 512], f32, name="pv", tag="pv")
                  for _ in range(2)]
            st.pv_tiles[(p, w)] = pv
            lag = None  # (kj, pr_tile, d_off)
            fi = 0  # fillers emitted

            def emit_pv(kj, pr, d_off):
                for h in range(2):
                    hl = 2 * p + h
                    nc.tensor.matmul(
                        pv[h][:, d_off:512],
                        st.v_t[kj][:, 65 * hl:65 * hl + 65],
                        pr[:, h, d_off:512],
                        start=(kj == 0),
                        stop=(kj == nkj - 1),
                    )

            for kj in range(nkj):
                c, ci = divmod(kj, 2)
                if ci == 0:
                    pump_erb(st.erb_cursor[(p, w, c)])
                d_off = max(0, (kj - 4 * w) * 128)
                sc = sc_ps.tile([128, 2, 512], f32, name="sc", tag="sc")
                for h in range(2):
                    rows = slice(64 * h, 64 * h + 64)
                    nc.tensor.matmul(
                        sc[:, h, d_off:512],
                        kT[rows, kj * 128:(kj + 1) * 128],
                        qT[rows, q0 + d_off:q0 + 512],
                        start=True, stop=True,
                        tile_position=(64 * h, 0),
                    )
                esc = esc_pool.tile([128, 2, 512], bf, name="esc", tag="esc")
                nc.scalar.activation(
                    esc[:, :, d_off:512], sc[:, :, d_off:512], EXP)
                erb_t, qoff = erb_tiles[(p, w, c)]
                pr = pr_pool.tile([128, 2, 512], bf, name="pr", tag="pr")
                nc.vector.tensor_mul(
                    pr[:, :, d_off:512],
                    esc[:, :, d_off:512],
                    erb_t[:, ci, :, d_off - qoff:512 - qoff])
                # fillers + lagged PV
                target = (kj + 1) * len(fillers) // nkj
                while fi < target:
                    fillers[fi]()
                    fi += 1
                if hooks and kj in hooks:
                    hooks[kj]()
                if lag is not None:
                    emit_pv(*lag)
                lag = (kj, pr, d_off)
            emit_pv(*lag)
            while fi < len(fillers):
                fillers[fi]()
                fi += 1
            # stage unnormalized PV to SBUF (f32, includes denominator row 64)
            for h in range(2):
                pvu = pvu_pool.tile([65, 512], f32, name="pvu", tag="pvu")
                nc.vector.tensor_copy(pvu[:], pv[h][:])
                st.pvu_tiles[(p, w, h)] = pvu

        def mk_norm_recip(p, w, den2, box):
            """DVE half of the normalization: 1/den -> bf16 row on part 0."""
            def emit():
                rinv = den_pool.tile([2, 512], f32, name="rinv", tag="rinv")
                nc.vector.reciprocal_approx_fast(rinv[:], den2[:])
                rinvbf = den_pool.tile([2, 512], bf, name="rinvbf", tag="rinvbf")
                nc.vector.tensor_copy(rinvbf[:], rinv[:])
                # matmul operands must start at a 32-aligned partition; flatten
                # the 2 rows into one partition-0 row via SBUF->SBUF DMA
                rrow = den_pool.tile([1, 2, 512], bf, name="rrow", tag="rrow")
                nc.sync.dma_start(rrow[:], rinvbf[:])
                box.append(rrow)
            return emit

        def mk_norm_apply(p, w, box):
            """PE+DVE half: broadcast 1/den across partitions, apply."""
            def emit():
                rrow = box.pop()
                rbc = aux_ps.tile([128, 512], f32, name="rbc", tag="aux")
                for h in range(2):
                    nc.tensor.matmul(
                        rbc[64 * h:64 * h + 64, :],
                        ones64[:],
                        rrow[0:1, h, :],
                        start=True, stop=True,
                        tile_position=(0, 64 * h),
                    )
                for h in range(2):
                    nc.vector.tensor_mul(
                        st.attnT_t[p][64 * h:64 * h + 64,
                                      512 * w:512 * (w + 1)],
                        st.pvu_tiles[(p, w, h)][0:64, :],
                        rbc[64 * h:64 * h + 64, :])
            return emit

        def mk_den_gather(p, w, den2):
            def emit():
                for h in range(2):
                    nc.sync.dma_start(
                        den2[h:h + 1, :],
                        st.pvu_tiles[(p, w, h)][64:65, :])
            return emit

        # erb consumption cursors (chunk index in erb_sched order)
        st.erb_cursor = {}
        ci_ = 0
        for w in range(NW):
            for p in range(2):
                for c in range(2 * w + 2):
                    st.erb_cursor[(p, w, c)] = ci_
                    ci_ += 1
        st.erb_consumed = 0
        st.pv_tiles = {}
        st.pvu_tiles = {}

        # ---------------- the schedule ----------------
        # window-0 erb ahead of the bulk x loads; later xt slices are
        # emitted as fillers at their point of need so they never
        # head-block just-in-time erb chunks on the DMA queue
        while st.erb_emitted < 4:
            erb_sched[st.erb_emitted][1]()
            st.erb_emitted += 1
        mk_xt_dma(1)()
        # prologue: q/k projections for pair0 queries+keys 0:512, v si 0
        mk_qk_unit(0, 0)()
        mk_qk_unit(1, 0)()
        mk_v_unit(0)()

        den2s = {}
        boxes = {}

        def gather(p, w):
            den2s[(p, w)] = den_pool.tile([2, 512], f32, name="den2",
                                          tag="den2")
            boxes[(p, w)] = []
            mk_den_gather(p, w, den2s[(p, w)])()

        def norm_hooks(p, w, nkj_next):
            return {1: mk_norm_recip(p, w, den2s[(p, w)], boxes[(p, w)]),
                    min(4, nkj_next - 1): mk_norm_apply(p, w, boxes[(p, w)])}

        # window sequence with per-window filler lists
        emit_window(0, 0, [mk_v_unit(1), mk_v_unit(2), mk_v_unit(3),
                           mk_qk_unit(2, 0), mk_qk_unit(3, 0)])
        gather(0, 0)
        emit_window(1, 0, [mk_qk_unit(0, 1), mk_qk_unit(1, 1)],
                    hooks=norm_hooks(0, 0, 4))
        gather(1, 0)

        emit_window(0, 1, [mk_xt_dma(2), mk_wo_dma(),
                           mk_v_unit(4), mk_v_unit(5), mk_v_unit(6),
                           mk_v_unit(7), mk_qk_unit(2, 1), mk_qk_unit(3, 1)],
                    hooks=norm_hooks(1, 0, 8))
        gather(0, 1)
        emit_window(1, 1, [mk_xt_dma(3), mk_qk_unit(0, 2), mk_qk_unit(1, 2)]
                    + [mk_out_unit(si, e2) for si in (0, 1) for e2 in (0, 1)],
                    hooks=norm_hooks(0, 1, 8))
        gather(1, 1)

        emit_window(0, 2, [mk_v_unit(8), mk_v_unit(9), mk_v_unit(10),
                           mk_v_unit(11), mk_qk_unit(2, 2), mk_qk_unit(3, 2)]
                    + [mk_out_unit(si, e2) for si in (2, 3) for e2 in (0, 1)],
                    hooks=norm_hooks(1, 1, 12))
        gather(0, 2)
        emit_window(1, 2, [mk_qk_unit(0, 3), mk_qk_unit(1, 3)]
                    + [mk_out_unit(si, e2) for si in (4, 5) for e2 in (0, 1)],
                    hooks=norm_hooks(0, 2, 12))
        gather(1, 2)

        emit_window(0, 3, [mk_v_unit(12), mk_v_unit(13), mk_v_unit(14),
                           mk_v_unit(15), mk_qk_unit(2, 3), mk_qk_unit(3, 3)]
                    + [mk_out_unit(si, e2) for si in (6, 7) for e2 in (0, 1)],
                    hooks=norm_hooks(1, 2, 16))
        gather(0, 3)
        emit_window(1, 3, [mk_out_unit(si, e2, "dve") for si in (8, 9)
                           for e2 in (0, 1)],
                    hooks=norm_hooks(0, 3, 16))
        gather(1, 3)
        # PE cover for the final norm chain latency
        mk_norm_recip(1, 3, den2s[(1, 3)], boxes[(1, 3)])()
        for si in (10, 11):
            for e2 in (0, 1):
                mk_out_unit(si, e2, "dve")()
        mk_norm_apply(1, 3, boxes[(1, 3)])()
        for si in (12, 13, 14, 15):
            for e2 in (0, 1):
                mk_out_unit(si, e2, "dve")()

    nc.compile()
    return nc


_PROGRAM_CACHE = {}


def _get_program(has_bqk, has_bv):
    key = (has_bqk, has_bv)
    if key not in _PROGRAM_CACHE:
        _PROGRAM_CACHE[key] = _build_program(has_bqk, has_bv)
    return _PROGRAM_CACHE[key]


_last_results = None  # BassKernelResults of the most recent run (for test.py)


def kernel(x, rel_bias, w_qkv, b_qkv, w_out, b_out, *, trace=False):
    global _last_results
    _install_ntff_hook()
    _enable_ldw_opt()
    from concourse.bass_utils import run_bass_kernel_spmd

    x = np.asarray(x, dtype=np.float32)
    rel_bias = np.asarray(rel_bias, dtype=np.float32)
    w_qkv = np.asarray(w_qkv, dtype=np.float32)
    b_qkv = np.asarray(b_qkv, dtype=np.float32)
    w_out = np.asarray(w_out, dtype=np.float32)
    b_out = np.asarray(b_out, dtype=np.float32)

    wq = w_qkv[:, 0:D]
    wk = w_qkv[:, D:2 * D]
    wv = w_qkv[:, 2 * D:3 * D]
    bq, bk, bv = b_qkv[0:D], b_qkv[D:2 * D], b_qkv[2 * D:3 * D]
    has_bqk = bool(np.any(bq)) or bool(np.any(bk))
    has_bv = bool(np.any(bv))

    nc = _get_program(has_bqk, has_bv)

    sc = 1.0 / math.sqrt(HD)  # folded into the q projection
    xT = [np.ascontiguousarray(x[b].T).astype(_BF16) for b in range(B)]
    tri = np.triu(np.ones((S, S), dtype=np.float32))  # [kj, qi]: qi >= kj

    in_maps = []
    for c in range(NCORES):
        b, hg = divmod(c, 4)
        hs = [4 * hg + i for i in range(HPC)]

        # wqk columns: [q_h0 | q_h1 | k_h0 | k_h1 | q_h2 | q_h3 | k_h2 | k_h3]
        cols = []
        bqk_rows = []
        for pair in range(2):
            h0, h1 = hs[2 * pair], hs[2 * pair + 1]
            cols += [wq[:, HD * h0:HD * (h0 + 1)] * sc,
                     wq[:, HD * h1:HD * (h1 + 1)] * sc]
            bqk_rows.append(np.concatenate(
                [bq[HD * h0:HD * (h0 + 1)], bq[HD * h1:HD * (h1 + 1)]]) * sc)
            cols += [wk[:, HD * h0:HD * (h0 + 1)],
                     wk[:, HD * h1:HD * (h1 + 1)]]
            bqk_rows.append(np.concatenate(
                [bk[HD * h0:HD * (h0 + 1)], bk[HD * h1:HD * (h1 + 1)]]))
        wqk_c = np.concatenate(cols, axis=1).astype(_BF16)
        bqk_c = np.stack(bqk_rows).astype(_BF16)

        wv_c = np.zeros((D, 260), dtype=np.float32)
        bv_c = np.zeros((1, 260), dtype=np.float32)
        for i, h in enumerate(hs):
            wv_c[:, 65 * i:65 * i + 64] = wv[:, HD * h:HD * (h + 1)]
            bv_c[0, 65 * i:65 * i + 64] = bv[HD * h:HD * (h + 1)]

        erb_c = np.empty((HPC, S, S), dtype=_BF16)
        for i, h in enumerate(hs):
            erb_c[i] = (np.exp(rel_bias[h].T) * tri).astype(_BF16)

        in_maps.append({
            "xT": xT[b],
            "wqk": wqk_c,
            "wv": wv_c.astype(_BF16),
            "bqk": bqk_c,
            "bv": bv_c.astype(_BF16),
            "erb": erb_c,
            "wo": np.ascontiguousarray(
                w_out[256 * hg:256 * (hg + 1)].reshape(2, 128, D)).astype(_BF16),
        })

    res = run_bass_kernel_spmd(nc, in_maps, list(range(NCORES)), trace=trace)
    _last_results = res

    out = np.zeros((B, S, D), dtype=np.float32)
    for c in range(NCORES):
        out[c // 4] += np.asarray(res.results[c]["out"], dtype=np.float32)
    out += b_out
    return out


# revision 25
# speedup vs baseline: 1.0470x; 1.0470x over previous
"""Causal multi-head attention with relative position bias on 8 Trainium2
NeuronCores.

Problem (full shapes): x[2,2048,1024], rel_bias[16,2048,2048],
w_qkv[1024,3072], b_qkv[3072], w_out[1024,1024], b_out[1024].

Sharding: core = (batch, head-group): 2 batches x 4 head-groups of 4 heads.
Each core computes q/k/v projections for its 4 heads, causal attention with
rel-bias, and a partial output projection through its heads' rows of w_out.
Host sums the 4 partial outputs per batch (the tensor-parallel reduce) and
adds b_out.

Device kernel design notes:
- Scores are computed TRANSPOSED (scoresT[kj,qi] = k.q) so no on-chip
  transposes are needed anywhere: softmax reduction over keys becomes a
  matmul contraction, handled by appending a ones-column to V; the PV matmul
  directly produces the transposed attention output that the out-projection
  needs as its stationary operand.
- exp(score + bias) = exp(score) * exp(bias): host precomputes exp(rel_biasT)
  in bf16 with the causal mask baked in as exact zeros. ACT does a pure exp
  straight from PSUM; DVE multiplies two bf16 SBUF operands at 2x rate.
- Queries are processed in 512-wide windows (4 per core); each window's
  score/exp/mul/PV tiles are trimmed at the causal diagonal.
- Both heads of a pair share one [128,2,512] score PSUM tile so exp and the
  erb multiply are ONE instruction per (window, key-block) - halving the
  per-instruction overhead on ACT and DVE.
- The PE instruction stream is statically interleaved: qk/v projection units
  and out-projection units are emitted as fillers inside the attention
  windows so the tensor engine never idles (keeps the HAM clock gate at
  2.4 GHz) while ACT/DVE drain the softmax work.
- Normalization is deferred and batched: unnormalized PV output is staged to
  SBUF (f32), denominator rows are DMA-gathered into a [4,512] tile, ONE
  reciprocal_approx_fast computes 1/den for the whole window round, a pair of
  K=1 matmuls broadcasts it across partitions, and DVE applies it while the
  out-projection of the previous round runs on the PE.
"""

import math
import sys
import types
from collections import deque
from contextlib import ExitStack

import ml_dtypes
import numpy as np

B, S, D = 2, 2048, 1024
NH, HD = 16, 64
NCORES = 8
HPC = 4  # heads per core (2 pairs)

_BF16 = ml_dtypes.bfloat16

KC = D // 128   # 8 contraction chunks for the projections
NW = S // 512   # 4 query windows
NSC = S // 128  # 16 s-chunks


def _install_ntff_hook():
    """concourse.bass_utils imports antenv.axon_hooks for NTFF tracing under
    axon; this container's antenv lacks that module. Provide it, backed by
    the ctypes hook from trn_agent_boot (if present)."""
    if "antenv.axon_hooks" in sys.modules:
        return
    try:
        import antenv
    except ImportError:
        return
    mod = types.ModuleType("antenv.axon_hooks")
    mod._hook = None
    mod.set_axon_ntff_profile_hook = lambda h: setattr(mod, "_hook", h)
    mod.get_axon_ntff_profile_hook = lambda: mod._hook
    sys.modules["antenv.axon_hooks"] = mod
    antenv.axon_hooks = mod
    try:
        from trn_agent_boot.trn_boot import _ntff_profile_via_ctypes

        h = _ntff_profile_via_ctypes("/opt/axon/libaxon_pjrt.so")
        if h is not None:
            mod._hook = h
    except Exception:
        pass


_LDW_OPT_INSTALLED = False


def _enable_ldw_opt():
    """walrus ships with --enable-ldw-opt=false; flip it for this process
    (dedupes/hoists LDWEIGHTS). Gated by KERNEL_LDW_OPT=1."""
    global _LDW_OPT_INSTALLED
    if _LDW_OPT_INSTALLED:
        return
    _LDW_OPT_INSTALLED = True
    import os
    if os.environ.get("KERNEL_LDW_OPT", "0") != "1":
        return
    import concourse.bass_utils as bu
    orig = bu.run_command

    def patched(argv, **kwargs):
        argv = ["--enable-ldw-opt=true" if a == "--enable-ldw-opt=false" else a
                for a in argv]
        return orig(argv, **kwargs)

    bu.run_command = patched


def _build_program(has_bqk: bool, has_bv: bool):
    import concourse.tile as tile
    from concourse import bacc, mybir

    bf = mybir.dt.bfloat16
    f32 = mybir.dt.float32
    EXP = mybir.ActivationFunctionType.Exp

    nc = bacc.Bacc("TRN2", target_bir_lowering=False, debug=False,
                   num_devices=NCORES)

    d = types.SimpleNamespace()
    d.xT = nc.dram_tensor("xT", [D, S], bf, kind="ExternalInput").ap()
    d.wqk = nc.dram_tensor("wqk", [D, 512], bf, kind="ExternalInput").ap()
    d.wv = nc.dram_tensor("wv", [D, 260], bf, kind="ExternalInput").ap()
    d.bqk = nc.dram_tensor("bqk", [4, 128], bf, kind="ExternalInput").ap()
    d.bv = nc.dram_tensor("bv", [1, 260], bf, kind="ExternalInput").ap()
    d.erb = nc.dram_tensor("erb", [HPC, S, S], bf, kind="ExternalInput").ap()
    d.wo = nc.dram_tensor("wo", [2, 128, D], bf, kind="ExternalInput").ap()
    d.out = nc.dram_tensor("out", [S, D], bf, kind="ExternalOutput").ap()

    # erb source view: p(row within key block), c(key block), h(local head), q
    erb_v = d.erb.rearrange("h (c p) q -> p c h q", p=128)

    st = types.SimpleNamespace()

    with tile.TileContext(nc) as tc, ExitStack() as ctx:
        # ---------------- SBUF pools ----------------
        xt_pool = ctx.enter_context(tc.tile_pool(name="xt", bufs=1))
        wqk_pool = ctx.enter_context(tc.tile_pool(name="wqk", bufs=1))
        wv_pool = ctx.enter_context(tc.tile_pool(name="wv", bufs=1))
        wo_pool = ctx.enter_context(tc.tile_pool(name="wo", bufs=1))
        const_pool = ctx.enter_context(tc.tile_pool(name="consts", bufs=1))
        qkT_pool = ctx.enter_context(tc.tile_pool(name="qkT", bufs=4))
        v_pool = ctx.enter_context(tc.tile_pool(name="vsb", bufs=NSC))
        attnT_pool = ctx.enter_context(tc.tile_pool(name="attnT", bufs=2))
        erb_pool = ctx.enter_context(tc.tile_pool(name="erb", bufs=13))
        esc_pool = ctx.enter_context(tc.tile_pool(name="esc", bufs=4))
        pr_pool = ctx.enter_context(tc.tile_pool(name="prob", bufs=5))
        pvu_pool = ctx.enter_context(tc.tile_pool(name="pvu", bufs=6))
        den_pool = ctx.enter_context(tc.tile_pool(name="den", bufs=2))
        osb_pool = ctx.enter_context(tc.tile_pool(name="osb", bufs=4))
        # ---------------- PSUM pools ----------------
        sc_ps = ctx.enter_context(tc.tile_pool(name="sc_ps", bufs=2, space="PSUM"))
        pv_ps = ctx.enter_context(tc.tile_pool(name="pv_ps", bufs=2, space="PSUM"))
        aux_ps = ctx.enter_context(tc.tile_pool(name="aux_ps", bufs=2, space="PSUM"))

        # ---------------- constants ----------------
        ones_row = const_pool.tile([1, 512], bf)
        nc.gpsimd.memset(ones_row[:], 1.0)
        ones64 = const_pool.tile([1, 64], bf)
        nc.gpsimd.memset(ones64[:], 1.0)

        # ---------------- input DMAs (consolidated, critical-path first) ----
        xT_v = d.xT.rearrange("(k p) q -> p k q", p=128)
        wqk_v = d.wqk.rearrange("(k p) c -> p k c", p=128)
        wv_v = d.wv.rearrange("(k p) c -> p k c", p=128)
        wo_v = d.wo.rearrange("w p d -> p w d")

        wqk_big = wqk_pool.tile([128, KC, 512], bf)
        nc.sync.dma_start(wqk_big[:, :, 0:256], wqk_v[:, :, 0:256])
        st.wqk_t = [wqk_big[:, k, :] for k in range(KC)]
        xt_big = xt_pool.tile([128, KC, S], bf)
        nc.sync.dma_start(xt_big[:, 0:4, 0:512], xT_v[:, 0:4, 0:512])
        nc.sync.dma_start(xt_big[:, 4:KC, 0:512], xT_v[:, 4:KC, 0:512])
        nc.sync.dma_start(wqk_big[:, :, 256:512], wqk_v[:, :, 256:512])
        st.xt_t = [xt_big[:, k, :] for k in range(KC)]
        wv_big = wv_pool.tile([128, KC, 260], bf)
        nc.sync.dma_start(wv_big[:], wv_v[:])
        st.wv_t = [wv_big[:, k, :] for k in range(KC)]

        def mk_xt_dma(s4):
            def emit():
                nc.sync.dma_start(
                    xt_big[:, :, s4 * 512:(s4 + 1) * 512],
                    xT_v[:, :, s4 * 512:(s4 + 1) * 512])
            return emit

        wo_big = wo_pool.tile([128, 2, D], bf)
        st.wo_t = [wo_big[:, p, :] for p in range(2)]

        def mk_wo_dma():
            def emit():
                nc.sync.dma_start(wo_big[:], wo_v[:])
            return emit

        if has_bqk:
            st.bqk_sb = []
            for m in range(4):
                t = const_pool.tile([1, 128], bf, name=f"bqk{m}", tag=f"bqk{m}")
                nc.sync.dma_start(t[:], d.bqk[m:m + 1, :])
                st.bqk_sb.append(t)
        if has_bv:
            st.bv_sb = const_pool.tile([1, 260], bf)
            nc.sync.dma_start(st.bv_sb[:], d.bv[:])

        st.qkT_t = [qkT_pool.tile([128, S], bf, name=f"qkT{m}", tag="qkT")
                    for m in range(4)]
        st.v_t = [v_pool.tile([128, 260], bf, name=f"v{si}", tag="vsb")
                  for si in range(NSC)]
        st.attnT_t = [attnT_pool.tile([128, S], bf, name=f"attnT{p}", tag="attnT")
                      for p in range(2)]

        # ---------------- erb chunk prefetch machinery ----------------
        # chunk list in consumption order; each covers 2 key-blocks x 2 heads
        erb_tiles = {}

        def erb_chunk_key(p, w, c):
            return (p, w, c)

        erb_sched = []  # (key, emit_fn)

        def mk_erb_emit(p, w, c):
            def emit():
                last = (c == 2 * w + 1)
                qoff = 256 if last else 0
                wd = 512 - qoff
                t = erb_pool.tile([128, 2, 2, 512], bf, name="erbt", tag="erbt")
                for h in range(2):
                    nc.sync.dma_start(
                        t[:, :, h, 0:wd],
                        erb_v[:, 2 * c:2 * c + 2, 2 * p + h,
                              512 * w + qoff:512 * (w + 1)])
                erb_tiles[(p, w, c)] = (t, qoff)
            return emit

        for w in range(NW):
            for p in range(2):
                for c in range(2 * w + 2):
                    erb_sched.append(((p, w, c), mk_erb_emit(p, w, c)))
        st.erb_emitted = 0

        ERB_LEAD = 10

        def pump_erb(consumed):
            # keep ERB_LEAD chunks issued ahead of the consumption cursor
            while (st.erb_emitted < len(erb_sched)
                   and st.erb_emitted < consumed + ERB_LEAD):
                erb_sched[st.erb_emitted][1]()
                st.erb_emitted += 1

        # ---------------- filler units ----------------
        def mk_qk_unit(m, s4):
            def emit():
                ps = aux_ps.tile([128, 512], f32, name="qkps", tag="aux")
                for k in range(KC):
                    nc.tensor.matmul(
                        ps[:],
                        st.wqk_t[k][:, m * 128:(m + 1) * 128],
                        st.xt_t[k][:, s4 * 512:(s4 + 1) * 512],
                        start=(k == 0),
                        stop=(k == KC - 1 and not has_bqk),
                    )
                if has_bqk:
                    nc.tensor.matmul(
                        ps[:], st.bqk_sb[m][:], ones_row[:, :],
                        start=False, stop=True,
                    )
                nc.vector.tensor_copy(
                    st.qkT_t[m][:, s4 * 512:(s4 + 1) * 512], ps[:])
            return emit

        def mk_v_unit(si):
            def emit():
                ps = aux_ps.tile([128, 512], f32, name="vps", tag="aux")
                for k in range(KC):
                    nc.tensor.matmul(
                        ps[:, 0:260],
                        st.xt_t[k][:, si * 128:(si + 1) * 128],
                        st.wv_t[k][:],
                        start=(k == 0),
                        stop=(k == KC - 1 and not has_bv),
                    )
                if has_bv:
                    nc.tensor.matmul(
                        ps[:, 0:260], ones_row[0:1, 0:128], st.bv_sb[:],
                        start=False, stop=True,
                    )
                nc.vector.tensor_copy(st.v_t[si][:], ps[:, 0:260])
                for h in range(HPC):
                    nc.gpsimd.memset(
                        st.v_t[si][:, 65 * h + 64:65 * h + 65], 1.0)
            return emit

        def mk_out_unit(si, e2, copy_eng="act"):
            def emit():
                ps = aux_ps.tile([128, 512], f32, name="ops", tag="aux")
                for p in range(2):
                    nc.tensor.matmul(
                        ps[:],
                        st.attnT_t[p][:, si * 128:(si + 1) * 128],
                        st.wo_t[p][:, e2 * 512:(e2 + 1) * 512],
                        start=(p == 0), stop=(p == 1),
                    )
                osb = osb_pool.tile([128, 512], bf, name="osb", tag="osb")
                if copy_eng == "act":
                    nc.scalar.copy(osb[:], ps[:])
                else:
                    nc.vector.tensor_copy(osb[:], ps[:])
                nc.sync.dma_start(
                    d.out[si * 128:(si + 1) * 128, e2 * 512:(e2 + 1) * 512],
                    osb[:])
            return emit

        # ---------------- attention window ----------------
        def emit_window(p, w, fillers, hooks=None):
            """One (pair, 512-query-window): scoresT -> exp -> *erb -> PV.
            `fillers`: list of closures to spread across the kj loop.
            `hooks`: dict kj -> closure, run after that kj's filler drain."""
            qT = st.qkT_t[2 * p]
            kT = st.qkT_t[2 * p + 1]
            nkj = 4 * w + 4
            q0 = 512 * w
            pv = [pv_ps.tile([65, 512], f32, name="pv", tag="pv")
                  for _ in range(2)]
            st.pv_tiles[(p, w)] = pv
            lags = deque()  # (kj, pr_tile, d_off), depth-2 pipeline
            fi = 0  # fillers emitted

            def emit_pv(kj, pr, d_off):
                for h in range(2):
                    hl = 2 * p + h
                    nc.tensor.matmul(
                        pv[h][:, d_off:512],
                        st.v_t[kj][:, 65 * hl:65 * hl + 65],
                        pr[:, h, d_off:512],
                        start=(kj == 0),
                        stop=(kj == nkj - 1),
                    )

            for kj in range(nkj):
                c, ci = divmod(kj, 2)
                if ci == 0:
                    pump_erb(st.erb_cursor[(p, w, c)])
                d_off = max(0, (kj - 4 * w) * 128)
                sc = sc_ps.tile([128, 2, 512], f32, name="sc", tag="sc")
                for h in range(2):
                    rows = slice(64 * h, 64 * h + 64)
                    nc.tensor.matmul(
                        sc[:, h, d_off:512],
                        kT[rows, kj * 128:(kj + 1) * 128],
                        qT[rows, q0 + d_off:q0 + 512],
                        start=True, stop=True,
                        tile_position=(64 * h, 0),
                    )
                esc = esc_pool.tile([128, 2, 512], bf, name="esc", tag="esc")
                nc.scalar.activation(
                    esc[:, :, d_off:512], sc[:, :, d_off:512], EXP)
                erb_t, qoff = erb_tiles[(p, w, c)]
                pr = pr_pool.tile([128, 2, 512], bf, name="pr", tag="pr")
                nc.vector.tensor_mul(
                    pr[:, :, d_off:512],
                    esc[:, :, d_off:512],
                    erb_t[:, ci, :, d_off - qoff:512 - qoff])
                # fillers + lagged PV
                target = (kj + 1) * len(fillers) // nkj
                while fi < target:
                    fillers[fi]()
                    fi += 1
                if hooks and kj in hooks:
                    hooks[kj]()
                if len(lags) >= 2:
                    emit_pv(*lags.popleft())
                lags.append((kj, pr, d_off))
            while lags:
                emit_pv(*lags.popleft())
            while fi < len(fillers):
                fillers[fi]()
                fi += 1
            # stage unnormalized PV to SBUF (f32, includes denominator row 64)
            for h in range(2):
                pvu = pvu_pool.tile([65, 512], f32, name="pvu", tag="pvu")
                nc.vector.tensor_copy(pvu[:], pv[h][:])
                st.pvu_tiles[(p, w, h)] = pvu

        def mk_norm_recip(p, w, den2, box):
            """DVE half of the normalization: 1/den -> bf16 row on part 0."""
            def emit():
                rinv = den_pool.tile([2, 512], f32, name="rinv", tag="rinv")
                nc.vector.reciprocal_approx_fast(rinv[:], den2[:])
                rinvbf = den_pool.tile([2, 512], bf, name="rinvbf", tag="rinvbf")
                nc.vector.tensor_copy(rinvbf[:], rinv[:])
                # matmul operands must start at a 32-aligned partition; flatten
                # the 2 rows into one partition-0 row via SBUF->SBUF DMA
                rrow = den_pool.tile([1, 2, 512], bf, name="rrow", tag="rrow")
                nc.sync.dma_start(rrow[:], rinvbf[:])
                box.append(rrow)
            return emit

        def mk_norm_apply(p, w, box):
            """PE+DVE half: broadcast 1/den across partitions, apply."""
            def emit():
                rrow = box.pop()
                rbc = aux_ps.tile([128, 512], f32, name="rbc", tag="aux")
                for h in range(2):
                    nc.tensor.matmul(
                        rbc[64 * h:64 * h + 64, :],
                        ones64[:],
                        rrow[0:1, h, :],
                        start=True, stop=True,
                        tile_position=(0, 64 * h),
                    )
                for h in range(2):
                    nc.vector.tensor_mul(
                        st.attnT_t[p][64 * h:64 * h + 64,
                                      512 * w:512 * (w + 1)],
                        st.pvu_tiles[(p, w, h)][0:64, :],
                        rbc[64 * h:64 * h + 64, :])
            return emit

        def mk_den_gather(p, w, den2):
            def emit():
                for h in range(2):
                    nc.sync.dma_start(
                        den2[h:h + 1, :],
                        st.pvu_tiles[(p, w, h)][64:65, :])
            return emit

        # erb consumption cursors (chunk index in erb_sched order)
        st.erb_cursor = {}
        ci_ = 0
        for w in range(NW):
            for p in range(2):
                for c in range(2 * w + 2):
                    st.erb_cursor[(p, w, c)] = ci_
                    ci_ += 1
        st.erb_consumed = 0
        st.pv_tiles = {}
        st.pvu_tiles = {}

        # ---------------- the schedule ----------------
        # window-0 erb ahead of the bulk x loads; later xt slices are
        # emitted as fillers at their point of need so they never
        # head-block just-in-time erb chunks on the DMA queue
        while st.erb_emitted < 4:
            erb_sched[st.erb_emitted][1]()
            st.erb_emitted += 1
        mk_xt_dma(1)()
        # prologue: q/k projections for pair0 queries+keys 0:512, v si 0
        mk_qk_unit(0, 0)()
        mk_qk_unit(1, 0)()
        mk_v_unit(0)()

        den2s = {}
        boxes = {}

        def gather(p, w):
            den2s[(p, w)] = den_pool.tile([2, 512], f32, name="den2",
                                          tag="den2")
            boxes[(p, w)] = []
            mk_den_gather(p, w, den2s[(p, w)])()

        def norm_hooks(p, w, nkj_next):
            return {1: mk_norm_recip(p, w, den2s[(p, w)], boxes[(p, w)]),
                    min(4, nkj_next - 1): mk_norm_apply(p, w, boxes[(p, w)])}

        # window sequence with per-window filler lists
        emit_window(0, 0, [mk_v_unit(1), mk_v_unit(2), mk_v_unit(3),
                           mk_qk_unit(2, 0), mk_qk_unit(3, 0)])
        gather(0, 0)
        emit_window(1, 0, [mk_qk_unit(0, 1), mk_qk_unit(1, 1)],
                    hooks=norm_hooks(0, 0, 4))
        gather(1, 0)

        emit_window(0, 1, [mk_xt_dma(2), mk_wo_dma(),
                           mk_v_unit(4), mk_v_unit(5), mk_v_unit(6),
                           mk_v_unit(7), mk_qk_unit(2, 1), mk_qk_unit(3, 1)],
                    hooks=norm_hooks(1, 0, 8))
        gather(0, 1)
        emit_window(1, 1, [mk_xt_dma(3), mk_qk_unit(0, 2), mk_qk_unit(1, 2)]
                    + [mk_out_unit(si, e2) for si in (0, 1) for e2 in (0, 1)],
                    hooks=norm_hooks(0, 1, 8))
        gather(1, 1)

        emit_window(0, 2, [mk_v_unit(8), mk_v_unit(9), mk_v_unit(10),
                           mk_v_unit(11), mk_qk_unit(2, 2), mk_qk_unit(3, 2)]
                    + [mk_out_unit(si, e2) for si in (2, 3) for e2 in (0, 1)],
                    hooks=norm_hooks(1, 1, 12))
        gather(0, 2)
        emit_window(1, 2, [mk_qk_unit(0, 3), mk_qk_unit(1, 3)]
                    + [mk_out_unit(si, e2) for si in (4, 5) for e2 in (0, 1)],
                    hooks=norm_hooks(0, 2, 12))
        gather(1, 2)

        emit_window(0, 3, [mk_v_unit(12), mk_v_unit(13), mk_v_unit(14),
                           mk_v_unit(15), mk_qk_unit(2, 3), mk_qk_unit(3, 3)]
                    + [mk_out_unit(si, e2) for si in (6, 7) for e2 in (0, 1)],
                    hooks=norm_hooks(1, 2, 16))
        gather(0, 3)
        emit_window(1, 3, [mk_out_unit(si, e2, "dve") for si in (8, 9)
                           for e2 in (0, 1)],
                    hooks=norm_hooks(0, 3, 16))
        gather(1, 3)
        # PE cover for the final norm chain latency
        mk_norm_recip(1, 3, den2s[(1, 3)], boxes[(1, 3)])()
        for si in (10, 11):
            for e2 in (0, 1):
                mk_out_unit(si, e2, "dve")()
        mk_norm_apply(1, 3, boxes[(1, 3)])()
        for si in (12, 13, 14, 15):
            for e2 in (0, 1):
                mk_out_unit(si, e2)()

    nc.compile()
    return nc


_PROGRAM_CACHE = {}


def _get_program(has_bqk, has_bv):
    key = (has_bqk, has_bv)
    if key not in _PROGRAM_CACHE:
        _PROGRAM_CACHE[key] = _build_program(has_bqk, has_bv)
    return _PROGRAM_CACHE[key]


_last_results = None  # BassKernelResults of the most recent run (for test.py)


def kernel(x, rel_bias, w_qkv, b_qkv, w_out, b_out, *, trace=False):
    global _last_results
    _install_ntff_hook()
    _enable_ldw_opt()
    from concourse.bass_utils import run_bass_kernel_spmd

    x = np.asarray(x, dtype=np.float32)
    rel_bias = np.asarray(rel_bias, dtype=np.float32)
    w_qkv = np.asarray(w_qkv, dtype=np.float32)
    b_qkv = np.asarray(b_qkv, dtype=np.float32)
    w_out = np.asarray(w_out, dtype=np.float32)
    b_out = np.asarray(b_out, dtype=np.float32)

    wq = w_qkv[:, 0:D]
    wk = w_qkv[:, D:2 * D]
    wv = w_qkv[:, 2 * D:3 * D]
    bq, bk, bv = b_qkv[0:D], b_qkv[D:2 * D], b_qkv[2 * D:3 * D]
    has_bqk = bool(np.any(bq)) or bool(np.any(bk))
    has_bv = bool(np.any(bv))

    nc = _get_program(has_bqk, has_bv)

    sc = 1.0 / math.sqrt(HD)  # folded into the q projection
    xT = [np.ascontiguousarray(x[b].T).astype(_BF16) for b in range(B)]
    tri = np.triu(np.ones((S, S), dtype=np.float32))  # [kj, qi]: qi >= kj

    in_maps = []
    for c in range(NCORES):
        b, hg = divmod(c, 4)
        hs = [4 * hg + i for i in range(HPC)]

        # wqk columns: [q_h0 | q_h1 | k_h0 | k_h1 | q_h2 | q_h3 | k_h2 | k_h3]
        cols = []
        bqk_rows = []
        for pair in range(2):
            h0, h1 = hs[2 * pair], hs[2 * pair + 1]
            cols += [wq[:, HD * h0:HD * (h0 + 1)] * sc,
                     wq[:, HD * h1:HD * (h1 + 1)] * sc]
            bqk_rows.append(np.concatenate(
                [bq[HD * h0:HD * (h0 + 1)], bq[HD * h1:HD * (h1 + 1)]]) * sc)
            cols += [wk[:, HD * h0:HD * (h0 + 1)],
                     wk[:, HD * h1:HD * (h1 + 1)]]
            bqk_rows.append(np.concatenate(
                [bk[HD * h0:HD * (h0 + 1)], bk[HD * h1:HD * (h1 + 1)]]))
        wqk_c = np.concatenate(cols, axis=1).astype(_BF16)
        bqk_c = np.stack(bqk_rows).astype(_BF16)

        wv_c = np.zeros((D, 260), dtype=np.float32)
        bv_c = np.zeros((1, 260), dtype=np.float32)
        for i, h in enumerate(hs):
            wv_c[:, 65 * i:65 * i + 64] = wv[:, HD * h:HD * (h + 1)]
            bv_c[0, 65 * i:65 * i + 64] = bv[HD * h:HD * (h + 1)]

        erb_c = np.empty((HPC, S, S), dtype=_BF16)
        for i, h in enumerate(hs):
            erb_c[i] = (np.exp(rel_bias[h].T) * tri).astype(_BF16)

        in_maps.append({
            "xT": xT[b],
            "wqk": wqk_c,
            "wv": wv_c.astype(_BF16),
            "bqk": bqk_c,
            "bv": bv_c.astype(_BF16),
            "erb": erb_c,
            "wo": np.ascontiguousarray(
                w_out[256 * hg:256 * (hg + 1)].reshape(2, 128, D)).astype(_BF16),
        })

    res = run_bass_kernel_spmd(nc, in_maps, list(range(NCORES)), trace=trace)
    _last_results = res

    out = np.zeros((B, S, D), dtype=np.float32)
    for c in range(NCORES):
        out[c // 4] += np.asarray(res.results[c]["out"], dtype=np.float32)
    out += b_out
    return out
